# revision 14
# baseline (speedup 1.0000x reference)
"""BSplineKAN layer kernel for 8 Trainium2 NeuronCores.

Math
----
The reference computes, per element x = clip(x, -1, 1):
    y[n,o] = sum_{i,b} basis_b(x[n,i]) * coeff[o,i,b]  +  silu(x) @ w_base.T + bias
where basis is the 7-function clamped cubic B-spline basis on knots
{-1(x4), -0.5, 0, 0.5, 1(x4)}.  A quirk of the reference recurrence: at
x == 1.0 exactly (all clamped x >= 1 inputs) the basis row is all ZERO.

On [-1, 1) the basis functions are C^2 piecewise cubics with breakpoints at
+-0.5; we represent them exactly in a two-window local feature basis: for
each half H in {L: [-1,0), R: [0,1)} with center c_H = -+0.5, u = x - c_H,
window mask m_H, and knot-side mask g_H:
    feats_H = [m_H, m_H*u, m_H*u^2, m_H*u^3, g_H*u^3]
All ten features vanish at x == 1 (masks exclude it), reproducing the
reference's edge behavior exactly.  basis_b = M[f,b] @ feats (M integer/48,
exact).  M is folded into coeff on the host and silu/w_base appended as an
11th feature, giving one fused fp16 matmul
    y[n,o] = sum_{i,f} F_f(x[n,i]) * W[f,i,o] + bias
with K = 11*1024 = 11264.  Features are local (|u| <= 0.5), so the
contraction has no large-term cancellation; fp16 operands with fp32 PSUM
accumulation give ~5e-4 scale-relative absmax error (validated vs fp64).
Masks are exact in fp16 and the u-chain rounds at most 3 times, so the
all-fp16 feature pipeline adds no measurable error.

Distribution: 4-way batch x 2-way d_out mesh over 8 cores.  Per core:
x arrives host-transposed as (1024, 2048) fp32 (transposing on host is part
of sharding and keeps TensorE free of transposes), W-shard (11264, 512)
fp16 stays resident in SBUF, output (2048, 512) fp32.  Features are
computed on DVE (fp16 chain, 2x/4x modes) + ACT (affine/square/silu), and
TensorE runs back-to-back 88-tile K-accumulations into PSUM.
"""

import numpy as np

# ---- problem constants (hardcoded per contract) ----
N_FULL, D_IN, D_OUT = 8192, 1024, 1024
MESH_N, MESH_O = 4, 2                 # 4-way batch x 2-way d_out
N_SHARD = N_FULL // MESH_N            # 2048
O_SHARD = D_OUT // MESH_O             # 512
P = 128
NF = 11                               # 10 spline features + silu
IB = D_IN // P                        # 8 i-blocks
KT = IB * NF                          # 88 K-tiles
NCHUNK = 256                          # batch cols per pipeline chunk
NSUB = NCHUNK // P                    # 2
CHUNKS = N_SHARD // NCHUNK            # 8

# basis_b = sum_f feats_f * M[f, b];  feats order:
# [mL, mL*uL, mL*uL^2, mL*uL^3, gL*uL^3, mR, mR*uR, mR*uR^2, mR*uR^3, gR*uR^3]
_M48 = np.array([
    [0,    12,   28,   8,    0,    0,    0],
    [0,   -72,   24,   48,   0,    0,    0],
    [0,    144, -240,  96,   0,    0,    0],
    [-384, 672, -352,  64,   0,    0,    0],
    [384, -768,  576, -256,  64,   0,    0],
    [0,    0,    0,    8,    28,   12,   0],
    [0,    0,    0,   -48,  -24,   72,   0],
    [0,    0,    0,    96,  -240,  144,  0],
    [0,    0,   -64,   192, -224,  96,   0],
    [0,    0,    64,  -256,  576, -768,  384],
], dtype=np.float64)

_PROGRAM = None  # compiled Bass program, built once


def _build_program():
    import concourse.mybir as mybir
    import concourse.tile as tile
    from concourse import bacc

    f32 = mybir.dt.float32
    f16 = mybir.dt.float16
    Op = mybir.AluOpType
    Act = mybir.ActivationFunctionType

    nc = bacc.Bacc("TRN2", target_bir_lowering=False, debug=False)
    xt_d = nc.dram_tensor("xt", [D_IN, N_SHARD], f32, kind="ExternalInput").ap()
    w_d = nc.dram_tensor("wt", [KT * P, O_SHARD], f16, kind="ExternalInput").ap()
    b_d = nc.dram_tensor("biasb", [P, O_SHARD], f32, kind="ExternalInput").ap()
    y_d = nc.dram_tensor("y", [N_SHARD, O_SHARD], f32, kind="ExternalOutput").ap()

    with tile.TileContext(nc) as tc:
        with (
            tc.tile_pool(name="const", bufs=1) as const_pool,
            tc.tile_pool(name="wt", bufs=1) as wt_pool,
            tc.tile_pool(name="feat", bufs=2) as f_pool,
            tc.tile_pool(name="xc", bufs=2) as xc_pool,
            tc.tile_pool(name="tmp", bufs=2) as tmp_pool,
            tc.tile_pool(name="out", bufs=1) as out_pool,
            tc.tile_pool(name="pso", bufs=4, space="PSUM") as psum_out,
        ):
            bias_s = const_pool.tile([P, O_SHARD], f32)
            nc.sync.dma_start(bias_s[:], b_d[:])
            # tiny dummy activations up front so both ACT table sets load
            # concurrently with the initial DMAs instead of on the first
            # feature's critical path
            warm = const_pool.tile([P, 1], f32, name="warm")
            nc.gpsimd.memset(warm[:], 0.0)
            nc.scalar.activation(warm[:], warm[:], Act.Copy, bias=0.0)
            nc.scalar.activation(warm[:], warm[:], Act.Square)
            nc.scalar.activation(warm[:], warm[:], Act.Silu)
            b05 = const_pool.tile([P, 1], f32, name="b05")
            nc.gpsimd.memset(b05[:], 0.5)
            bm05 = const_pool.tile([P, 1], f32, name="bm05")
            nc.gpsimd.memset(bm05[:], -0.5)

            # chunk-0 x loads go first so features can start while Wt streams in
            xc0 = {}
            for ib in range(IB):
                t = xc_pool.tile([P, NCHUNK], f32, tag=f"xc_{ib}", name=f"xc_{ib}")
                nc.sync.dma_start(t[:], xt_d[ib * P:(ib + 1) * P, 0:NCHUNK])
                xc0[ib] = t

            wt = {}
            for ib in range(IB):
                for f in range(NF):
                    t = wt_pool.tile([P, O_SHARD], f16, tag=f"wt_{ib}_{f}",
                                     name=f"wt_{ib}_{f}")
                    r0 = (ib * NF + f) * P
                    nc.sync.dma_start(t[:], w_d[r0:r0 + P, :])
                    wt[ib, f] = t

            for chunk in range(CHUNKS):
                c0 = chunk * NCHUNK
                # -- load x^T slices, clamp, build fp16 features --
                F = {}
                for ib in range(IB):
                    if chunk == 0:
                        xcb = xc0[ib]
                    else:
                        xcb = xc_pool.tile([P, NCHUNK], f32, tag=f"xc_{ib}", name=f"xc_{ib}")
                        nc.sync.dma_start(xcb[:], xt_d[ib * P:(ib + 1) * P, c0:c0 + NCHUNK])
                    nc.vector.tensor_scalar(xcb[:], xcb[:], -1.0, 1.0, Op.max, Op.min)

                    def tmp(tag, w=2, bufs=2):
                        return tmp_pool.tile([P, w * NCHUNK], f16, tag=tag, name=tag, bufs=bufs)

                    def fpair(f):
                        # (128, 2*NCHUNK) tile holding K-tiles (ib, f) on the
                        # left half and (ib, f+5) on the right half
                        t = f_pool.tile([P, 2 * NCHUNK], f16, tag=f"F_{ib}_{f}",
                                        name=f"F_{ib}_{f}")
                        F[ib, f] = t
                        return t

                    N = NCHUNK
                    # cumulative masks on GpSimd (1-input ops run near line-rate there)
                    cA = tmp("cA", 1, 1); nc.gpsimd.tensor_scalar(cA[:], xcb[:], -0.5, None, Op.is_ge)
                    cB = tmp("cB", 1); nc.gpsimd.tensor_scalar(cB[:], xcb[:], 0.0, None, Op.is_ge)
                    cC = tmp("cC", 1, 1); nc.gpsimd.tensor_scalar(cC[:], xcb[:], 0.5, None, Op.is_ge)
                    cD = tmp("cD", 1, 1); nc.gpsimd.tensor_scalar(cD[:], xcb[:], 1.0, None, Op.is_ge)
                    # window masks (exact 0/1 in fp16): Fm = [mL | mR]
                    Fm = fpair(0)
                    nc.vector.tensor_scalar(Fm[:, :N], cB[:], -1.0, 1.0, Op.mult, Op.add)
                    nc.vector.tensor_tensor(Fm[:, N:], cB[:], cD[:], Op.subtract)
                    Gm = tmp("Gm")
                    nc.vector.tensor_tensor(Gm[:, :N], cA[:], cB[:], Op.subtract)
                    nc.vector.tensor_tensor(Gm[:, N:], cC[:], cD[:], Op.subtract)
                    # u-chain: ACT writes both halves from the same xcb
                    U = tmp("U")
                    nc.scalar.activation(U[:, :N], xcb[:], Act.Copy, bias=0.5)
                    nc.scalar.activation(U[:, N:], xcb[:], Act.Copy, bias=-0.5)
                    U2 = tmp("U2")
                    nc.scalar.activation(U2[:, :N], xcb[:], Act.Square, bias=b05[:])
                    nc.scalar.activation(U2[:, N:], xcb[:], Act.Square, bias=bm05[:])
                    U3 = tmp("U3")
                    nc.vector.tensor_tensor(U3[:], U2[:], U[:], Op.mult)
                    # windowed monomials: one 512-wide fp16 op per L/R pair
                    nc.vector.tensor_tensor(fpair(1)[:], Fm[:], U[:], Op.mult)
                    nc.vector.tensor_tensor(fpair(2)[:], Fm[:], U2[:], Op.mult)
                    nc.vector.tensor_tensor(fpair(3)[:], Fm[:], U3[:], Op.mult)
                    nc.vector.tensor_tensor(fpair(4)[:], Gm[:], U3[:], Op.mult)
                    fs = f_pool.tile([P, NCHUNK], f16, tag=f"F_{ib}_s", name=f"F_{ib}_s")
                    F[ib, 10] = fs
                    nc.scalar.activation(fs[:], xcb[:], Act.Silu)

                # -- matmuls: K-accumulate 88 tiles per 128-batch subtile --
                for ns in range(NSUB):
                    ps = psum_out.tile([P, O_SHARD], f32, tag="psout", name="psout")
                    k = 0
                    for ib in range(IB):
                        for f in range(NF):
                            if f == 10:
                                lhsT = F[ib, 10][:, ns * P:(ns + 1) * P]
                            elif f < 5:
                                lhsT = F[ib, f][:, ns * P:(ns + 1) * P]
                            else:
                                lhsT = F[ib, f - 5][:, NCHUNK + ns * P:NCHUNK + (ns + 1) * P]
                            nc.tensor.matmul(
                                ps[:], lhsT, wt[ib, f][:],
                                start=(k == 0), stop=(k == KT - 1))
                            k += 1
                    o = out_pool.tile([P, O_SHARD], f32, tag="out", name="outt")
                    nc.vector.tensor_tensor(o[:], ps[:], bias_s[:], Op.add)
                    r0 = c0 + ns * P
                    nc.sync.dma_start(y_d[r0:r0 + P, :], o[:])

    nc.compile()
    return nc


def _fold_weights(coeff, w_base):
    """Fold the feature->basis matrix into coeff; returns (K, D_OUT) fp16."""
    M = _M48 / 48.0
    c64 = np.asarray(coeff).astype(np.float64)
    # Wf[f, i, o] = sum_b M[f, b] * coeff[o, i, b]
    Wf = np.einsum('fb,oib->fio', M, c64)
    W11 = np.concatenate([Wf, np.asarray(w_base).astype(np.float64).T[None]], axis=0)  # (11, i, o)
    # pack K as (ib, f, p): row k = ib*(NF*P) + f*P + p  <->  W11[f, ib*P+p, o]
    Wt = W11.reshape(NF, IB, P, D_OUT).transpose(1, 0, 2, 3).reshape(KT * P, D_OUT)
    return Wt.astype(np.float16)


def kernel(x, coeff, w_base, bias):
    global _PROGRAM
    from concourse.bass_utils import run_bass_kernel_spmd

    if _PROGRAM is None:
        _PROGRAM = _build_program()
    nc = _PROGRAM

    x = np.asarray(x, dtype=np.float32)
    Wt = _fold_weights(coeff, w_base)
    bias = np.asarray(bias, dtype=np.float32)

    in_maps = []
    for core in range(8):
        cn, co = divmod(core, MESH_O)
        in_maps.append({
            "xt": np.ascontiguousarray(x[cn * N_SHARD:(cn + 1) * N_SHARD].T),
            "wt": np.ascontiguousarray(Wt[:, co * O_SHARD:(co + 1) * O_SHARD]),
            "biasb": np.ascontiguousarray(np.broadcast_to(
                bias[co * O_SHARD:(co + 1) * O_SHARD], (P, O_SHARD)).astype(np.float32)),
        })

    res = run_bass_kernel_spmd(nc, in_maps, list(range(8)))

    y = np.empty((N_FULL, D_OUT), dtype=np.float32)
    for core in range(8):
        cn, co = divmod(core, MESH_O)
        y[cn * N_SHARD:(cn + 1) * N_SHARD, co * O_SHARD:(co + 1) * O_SHARD] = \
            res.results[core]["y"]
    return y


# revision 20
# speedup vs baseline: 1.0450x; 1.0450x over previous
"""BSplineKAN layer kernel for 8 Trainium2 NeuronCores.

Math
----
The reference computes, per element x = clip(x, -1, 1):
    y[n,o] = sum_{i,b} basis_b(x[n,i]) * coeff[o,i,b]  +  silu(x) @ w_base.T + bias
where basis is the 7-function clamped cubic B-spline basis on knots
{-1(x4), -0.5, 0, 0.5, 1(x4)}.  A quirk of the reference recurrence: at
x == 1.0 exactly (all clamped x >= 1 inputs) the basis row is all ZERO.

On [-1, 1) the basis functions are C^2 piecewise cubics with breakpoints at
+-0.5; we represent them exactly in a two-window local feature basis: for
each half H in {L: [-1,0), R: [0,1)} with center c_H = -+0.5, u = x - c_H,
window mask m_H, and knot-side mask g_H:
    feats_H = [m_H, m_H*u, m_H*u^2, m_H*u^3, g_H*u^3]
All ten features vanish at x == 1 (masks exclude it), reproducing the
reference's edge behavior exactly.  basis_b = M[f,b] @ feats (M integer/48,
exact).  M is folded into coeff on the host and silu/w_base appended as an
11th feature, giving one fused fp16 matmul
    y[n,o] = sum_{i,f} F_f(x[n,i]) * W[f,i,o] + bias
with K = 11*1024 = 11264.  Features are local (|u| <= 0.5), so the
contraction has no large-term cancellation; fp16 operands with fp32 PSUM
accumulation give ~5e-4 scale-relative absmax error (validated vs fp64).
Masks are exact in fp16 and the u-chain rounds at most 3 times, so the
all-fp16 feature pipeline adds no measurable error.

Distribution: 4-way batch x 2-way d_out mesh over 8 cores.  Per core:
x arrives host-transposed as (1024, 2048) fp32 (transposing on host is part
of sharding and keeps TensorE free of transposes), W-shard (11264, 512)
fp16 stays resident in SBUF, output (2048, 512) fp32.  Features are
computed on DVE (fp16 chain, 2x/4x modes) + ACT (affine/square/silu), and
TensorE runs back-to-back 88-tile K-accumulations into PSUM.
"""

import numpy as np

# ---- problem constants (hardcoded per contract) ----
N_FULL, D_IN, D_OUT = 8192, 1024, 1024
MESH_N, MESH_O = 4, 2                 # 4-way batch x 2-way d_out
N_SHARD = N_FULL // MESH_N            # 2048
O_SHARD = D_OUT // MESH_O             # 512
P = 128
NF = 11                               # 10 spline features + silu
IB = D_IN // P                        # 8 i-blocks
KT = IB * NF                          # 88 K-tiles
NCHUNK = 256                          # batch cols per pipeline chunk
NSUB = NCHUNK // P                    # 2
CHUNKS = N_SHARD // NCHUNK            # 8

# basis_b = sum_f feats_f * M[f, b];  feats order:
# [mL, mL*uL, mL*uL^2, mL*uL^3, gL*uL^3, mR, mR*uR, mR*uR^2, mR*uR^3, gR*uR^3]
_M48 = np.array([
    [0,    12,   28,   8,    0,    0,    0],
    [0,   -72,   24,   48,   0,    0,    0],
    [0,    144, -240,  96,   0,    0,    0],
    [-384, 672, -352,  64,   0,    0,    0],
    [384, -768,  576, -256,  64,   0,    0],
    [0,    0,    0,    8,    28,   12,   0],
    [0,    0,    0,   -48,  -24,   72,   0],
    [0,    0,    0,    96,  -240,  144,  0],
    [0,    0,   -64,   192, -224,  96,   0],
    [0,    0,    64,  -256,  576, -768,  384],
], dtype=np.float64)

_PROGRAM = None  # compiled Bass program, built once


def _build_program():
    import concourse.mybir as mybir
    import concourse.tile as tile
    from concourse import bacc

    f32 = mybir.dt.float32
    f16 = mybir.dt.float16
    Op = mybir.AluOpType
    Act = mybir.ActivationFunctionType

    nc = bacc.Bacc("TRN2", target_bir_lowering=False, debug=False)
    xt_d = nc.dram_tensor("xt", [D_IN, N_SHARD], f32, kind="ExternalInput").ap()
    w_d = nc.dram_tensor("wt", [KT * P, O_SHARD], f16, kind="ExternalInput").ap()
    b_d = nc.dram_tensor("biasb", [P, O_SHARD], f32, kind="ExternalInput").ap()
    y_d = nc.dram_tensor("y", [N_SHARD, O_SHARD], f32, kind="ExternalOutput").ap()

    with tile.TileContext(nc) as tc:
        with (
            tc.tile_pool(name="const", bufs=1) as const_pool,
            tc.tile_pool(name="wt", bufs=1) as wt_pool,
            tc.tile_pool(name="feat", bufs=2) as f_pool,
            tc.tile_pool(name="xc", bufs=2) as xc_pool,
            tc.tile_pool(name="tmp", bufs=2) as tmp_pool,
            tc.tile_pool(name="out", bufs=1) as out_pool,
            tc.tile_pool(name="pso", bufs=4, space="PSUM") as psum_out,
        ):
            bias_s = const_pool.tile([P, O_SHARD], f32)
            nc.sync.dma_start(bias_s[:], b_d[:])
            # tiny dummy activations up front so both ACT table sets load
            # concurrently with the initial DMAs instead of on the first
            # feature's critical path
            warm = const_pool.tile([P, 1], f32, name="warm")
            nc.gpsimd.memset(warm[:], 0.0)
            nc.scalar.activation(warm[:], warm[:], Act.Copy, bias=0.0)
            nc.scalar.activation(warm[:], warm[:], Act.Square)
            nc.scalar.activation(warm[:], warm[:], Act.Silu)
            b05 = const_pool.tile([P, 1], f32, name="b05")
            nc.gpsimd.memset(b05[:], 0.5)
            bm05 = const_pool.tile([P, 1], f32, name="bm05")
            nc.gpsimd.memset(bm05[:], -0.5)

            # chunk-0 x load goes first so features can start while Wt streams in
            xt_r = xt_d.rearrange("(ib p) n -> p ib n", p=P)
            xc0 = xc_pool.tile([P, IB, NCHUNK], f32, tag="xc", name="xc0")
            nc.sync.dma_start(xc0[:], xt_r[:, :, 0:NCHUNK])

            # one DMA per ib-slab of 11 weight tiles: HWDGE charges per DMA
            # instruction, so batching is what keeps the queue off the
            # critical path during the initial weight stream
            wt = {}
            for ib in range(IB):
                t = wt_pool.tile([P, NF, O_SHARD], f16, tag=f"wt_{ib}", name=f"wt_{ib}")
                r0 = ib * NF * P
                nc.sync.dma_start(
                    t[:], w_d[r0:r0 + NF * P, :].rearrange("(f p) o -> p f o", p=P))
                wt[ib] = t

            for chunk in range(CHUNKS):
                c0 = chunk * NCHUNK
                # -- load x^T slices, clamp, build fp16 features --
                F = {}
                if chunk == 0:
                    xch = xc0
                else:
                    xch = xc_pool.tile([P, IB, NCHUNK], f32, tag="xc", name="xc")
                    nc.sync.dma_start(xch[:], xt_r[:, :, c0:c0 + NCHUNK])
                nc.vector.tensor_scalar(xch[:], xch[:], -1.0, 1.0, Op.max, Op.min)
                for ib in range(IB):
                    xcb = xch[:, ib]

                    def tmp(tag, w=2, bufs=2):
                        return tmp_pool.tile([P, w * NCHUNK], f16, tag=tag, name=tag, bufs=bufs)

                    def fpair(f):
                        # (128, 2*NCHUNK) tile holding K-tiles (ib, f) on the
                        # left half and (ib, f+5) on the right half
                        t = f_pool.tile([P, 2 * NCHUNK], f16, tag=f"F_{ib}_{f}",
                                        name=f"F_{ib}_{f}")
                        F[ib, f] = t
                        return t

                    N = NCHUNK
                    # cumulative masks on GpSimd (1-input ops run near line-rate there)
                    cB = tmp("cB", 1); nc.gpsimd.tensor_scalar(cB[:], xcb[:], 0.0, None, Op.is_ge)
                    cD = tmp("cD", 1, 1); nc.gpsimd.tensor_scalar(cD[:], xcb[:], 1.0, None, Op.is_ge)
                    # window masks (exact 0/1 in fp16): Fm = [mL | mR]
                    Fm = fpair(0)
                    nc.gpsimd.tensor_scalar(Fm[:, :N], xcb[:], 0.0, None, Op.is_lt)
                    nc.vector.tensor_tensor(Fm[:, N:], cB[:], cD[:], Op.subtract)
                    # u-chain: ACT writes both halves from the same xcb
                    U = tmp("U")
                    nc.scalar.activation(U[:, :N], xcb[:], Act.Copy, bias=0.5)
                    nc.scalar.activation(U[:, N:], xcb[:], Act.Copy, bias=-0.5)
                    U2 = tmp("U2")
                    nc.scalar.activation(U2[:, :N], xcb[:], Act.Square, bias=b05[:])
                    nc.scalar.activation(U2[:, N:], xcb[:], Act.Square, bias=bm05[:])
                    U3 = tmp("U3")
                    nc.vector.tensor_tensor(U3[:], U2[:], U[:], Op.mult)
                    # windowed monomials: one 512-wide fp16 op per L/R pair
                    nc.vector.tensor_tensor(fpair(1)[:], Fm[:], U[:], Op.mult)
                    nc.vector.tensor_tensor(fpair(2)[:], Fm[:], U2[:], Op.mult)
                    nc.vector.tensor_tensor(fpair(3)[:], Fm[:], U3[:], Op.mult)
                    # knot-side features: g_H * u^3 == m_H * relu(u^3), fused
                    nc.vector.scalar_tensor_tensor(fpair(4)[:], U3[:], 0.0, Fm[:],
                                                   Op.max, Op.mult)
                    fs = f_pool.tile([P, NCHUNK], f16, tag=f"F_{ib}_s", name=f"F_{ib}_s")
                    F[ib, 10] = fs
                    nc.scalar.activation(fs[:], xcb[:], Act.Silu)

                # -- matmuls: k-major over both 128-batch subtiles, so each
                # weight tile feeds two matmuls the moment its DMA lands --
                pss = [psum_out.tile([P, O_SHARD], f32, tag=f"psout{ns}",
                                     name=f"psout{ns}", bufs=2) for ns in range(NSUB)]
                k = 0
                for ib in range(IB):
                    for f in range(NF):
                        for ns in range(NSUB):
                            if f == 10:
                                lhsT = F[ib, 10][:, ns * P:(ns + 1) * P]
                            elif f < 5:
                                lhsT = F[ib, f][:, ns * P:(ns + 1) * P]
                            else:
                                lhsT = F[ib, f - 5][:, NCHUNK + ns * P:NCHUNK + (ns + 1) * P]
                            nc.tensor.matmul(
                                pss[ns][:], lhsT, wt[ib][:, f],
                                start=(k == 0), stop=(k == KT - 1))
                        k += 1
                for ns in range(NSUB):
                    o = out_pool.tile([P, O_SHARD], f32, tag="out", name="outt")
                    nc.vector.tensor_tensor(o[:], pss[ns][:], bias_s[:], Op.add)
                    r0 = c0 + ns * P
                    nc.sync.dma_start(y_d[r0:r0 + P, :], o[:])

    nc.compile()
    return nc


def _fold_weights(coeff, w_base):
    """Fold the feature->basis matrix into coeff; returns (K, D_OUT) fp16."""
    M = _M48 / 48.0
    c64 = np.asarray(coeff).astype(np.float64)
    # Wf[f, i, o] = sum_b M[f, b] * coeff[o, i, b]
    Wf = np.einsum('fb,oib->fio', M, c64)
    W11 = np.concatenate([Wf, np.asarray(w_base).astype(np.float64).T[None]], axis=0)  # (11, i, o)
    # pack K as (ib, f, p): row k = ib*(NF*P) + f*P + p  <->  W11[f, ib*P+p, o]
    Wt = W11.reshape(NF, IB, P, D_OUT).transpose(1, 0, 2, 3).reshape(KT * P, D_OUT)
    return Wt.astype(np.float16)


def kernel(x, coeff, w_base, bias):
    global _PROGRAM
    from concourse.bass_utils import run_bass_kernel_spmd

    if _PROGRAM is None:
        _PROGRAM = _build_program()
    nc = _PROGRAM

    x = np.asarray(x, dtype=np.float32)
    Wt = _fold_weights(coeff, w_base)
    bias = np.asarray(bias, dtype=np.float32)

    in_maps = []
    for core in range(8):
        cn, co = divmod(core, MESH_O)
        in_maps.append({
            "xt": np.ascontiguousarray(x[cn * N_SHARD:(cn + 1) * N_SHARD].T),
            "wt": np.ascontiguousarray(Wt[:, co * O_SHARD:(co + 1) * O_SHARD]),
            "biasb": np.ascontiguousarray(np.broadcast_to(
                bias[co * O_SHARD:(co + 1) * O_SHARD], (P, O_SHARD)).astype(np.float32)),
        })

    res = run_bass_kernel_spmd(nc, in_maps, list(range(8)))

    y = np.empty((N_FULL, D_OUT), dtype=np.float32)
    for core in range(8):
        cn, co = divmod(core, MESH_O)
        y[cn * N_SHARD:(cn + 1) * N_SHARD, co * O_SHARD:(co + 1) * O_SHARD] = \
            res.results[core]["y"]
    return y


# revision 21
# speedup vs baseline: 1.0568x; 1.0113x over previous
"""BSplineKAN layer kernel for 8 Trainium2 NeuronCores.

Math
----
The reference computes, per element x = clip(x, -1, 1):
    y[n,o] = sum_{i,b} basis_b(x[n,i]) * coeff[o,i,b]  +  silu(x) @ w_base.T + bias
where basis is the 7-function clamped cubic B-spline basis on knots
{-1(x4), -0.5, 0, 0.5, 1(x4)}.  A quirk of the reference recurrence: at
x == 1.0 exactly (all clamped x >= 1 inputs) the basis row is all ZERO.

On [-1, 1) the basis functions are C^2 piecewise cubics with breakpoints at
+-0.5; we represent them exactly in a two-window local feature basis: for
each half H in {L: [-1,0), R: [0,1)} with center c_H = -+0.5, u = x - c_H,
window mask m_H, and knot-side mask g_H:
    feats_H = [m_H, m_H*u, m_H*u^2, m_H*u^3, g_H*u^3]
All ten features vanish at x == 1 (masks exclude it), reproducing the
reference's edge behavior exactly.  basis_b = M[f,b] @ feats (M integer/48,
exact).  M is folded into coeff on the host and silu/w_base appended as an
11th feature, giving one fused fp16 matmul
    y[n,o] = sum_{i,f} F_f(x[n,i]) * W[f,i,o] + bias
with K = 11*1024 = 11264.  Features are local (|u| <= 0.5), so the
contraction has no large-term cancellation; fp16 operands with fp32 PSUM
accumulation give ~5e-4 scale-relative absmax error (validated vs fp64).
Masks are exact in fp16 and the u-chain rounds at most 3 times, so the
all-fp16 feature pipeline adds no measurable error.

Distribution: 4-way batch x 2-way d_out mesh over 8 cores.  Per core:
x arrives host-transposed as (1024, 2048) fp32 (transposing on host is part
of sharding and keeps TensorE free of transposes), W-shard (11264, 512)
fp16 stays resident in SBUF, output (2048, 512) fp32.  Features are
computed on DVE (fp16 chain, 2x/4x modes) + ACT (affine/square/silu), and
TensorE runs back-to-back 88-tile K-accumulations into PSUM.
"""

import numpy as np

# ---- problem constants (hardcoded per contract) ----
N_FULL, D_IN, D_OUT = 8192, 1024, 1024
MESH_N, MESH_O = 4, 2                 # 4-way batch x 2-way d_out
N_SHARD = N_FULL // MESH_N            # 2048
O_SHARD = D_OUT // MESH_O             # 512
P = 128
NF = 11                               # 10 spline features + silu
IB = D_IN // P                        # 8 i-blocks
KT = IB * NF                          # 88 K-tiles
NCHUNK = 256                          # batch cols per pipeline chunk
NSUB = NCHUNK // P                    # 2
CHUNKS = N_SHARD // NCHUNK            # 8

# basis_b = sum_f feats_f * M[f, b];  feats order:
# [mL, mL*uL, mL*uL^2, mL*uL^3, gL*uL^3, mR, mR*uR, mR*uR^2, mR*uR^3, gR*uR^3]
_M48 = np.array([
    [0,    12,   28,   8,    0,    0,    0],
    [0,   -72,   24,   48,   0,    0,    0],
    [0,    144, -240,  96,   0,    0,    0],
    [-384, 672, -352,  64,   0,    0,    0],
    [384, -768,  576, -256,  64,   0,    0],
    [0,    0,    0,    8,    28,   12,   0],
    [0,    0,    0,   -48,  -24,   72,   0],
    [0,    0,    0,    96,  -240,  144,  0],
    [0,    0,   -64,   192, -224,  96,   0],
    [0,    0,    64,  -256,  576, -768,  384],
], dtype=np.float64)

_PROGRAM = None  # compiled Bass program, built once


def _build_program():
    import concourse.mybir as mybir
    import concourse.tile as tile
    from concourse import bacc

    f32 = mybir.dt.float32
    f16 = mybir.dt.float16
    Op = mybir.AluOpType
    Act = mybir.ActivationFunctionType

    nc = bacc.Bacc("TRN2", target_bir_lowering=False, debug=False)
    xt_d = nc.dram_tensor("xt", [D_IN, N_SHARD], f32, kind="ExternalInput").ap()
    w_d = nc.dram_tensor("wt", [KT * P, O_SHARD], f16, kind="ExternalInput").ap()
    b_d = nc.dram_tensor("biasb", [P, O_SHARD], f32, kind="ExternalInput").ap()
    y_d = nc.dram_tensor("y", [N_SHARD, O_SHARD], f32, kind="ExternalOutput").ap()

    with tile.TileContext(nc) as tc:
        with (
            tc.tile_pool(name="const", bufs=1) as const_pool,
            tc.tile_pool(name="wt", bufs=1) as wt_pool,
            tc.tile_pool(name="feat", bufs=2) as f_pool,
            tc.tile_pool(name="xc", bufs=2) as xc_pool,
            tc.tile_pool(name="tmp", bufs=2) as tmp_pool,
            tc.tile_pool(name="out", bufs=1) as out_pool,
            tc.tile_pool(name="pso", bufs=4, space="PSUM") as psum_out,
        ):
            bias_s = const_pool.tile([P, O_SHARD], f32)
            nc.sync.dma_start(bias_s[:], b_d[:])
            # tiny dummy activations up front so both ACT table sets load
            # concurrently with the initial DMAs instead of on the first
            # feature's critical path
            warm = const_pool.tile([P, 1], f32, name="warm")
            nc.gpsimd.memset(warm[:], 0.0)
            nc.scalar.activation(warm[:], warm[:], Act.Copy, bias=0.0)
            nc.scalar.activation(warm[:], warm[:], Act.Square)
            nc.scalar.activation(warm[:], warm[:], Act.Silu)
            b05 = const_pool.tile([P, 1], f32, name="b05")
            nc.gpsimd.memset(b05[:], 0.5)
            bm05 = const_pool.tile([P, 1], f32, name="bm05")
            nc.gpsimd.memset(bm05[:], -0.5)

            # chunk-0 x load goes first so features can start while Wt streams in
            xt_r = xt_d.rearrange("(ib p) n -> p ib n", p=P)
            xc0 = xc_pool.tile([P, IB, NCHUNK], f32, tag="xc", name="xc0")
            nc.sync.dma_start(xc0[:], xt_r[:, :, 0:NCHUNK])

            # one DMA per ib-slab of 11 weight tiles: HWDGE charges per DMA
            # instruction, so batching is what keeps the queue off the
            # critical path during the initial weight stream
            wt = {}
            for ib in range(IB):
                t = wt_pool.tile([P, NF, O_SHARD], f16, tag=f"wt_{ib}", name=f"wt_{ib}")
                r0 = ib * NF * P
                nc.sync.dma_start(
                    t[:], w_d[r0:r0 + NF * P, :].rearrange("(f p) o -> p f o", p=P))
                wt[ib] = t

            for chunk in range(CHUNKS):
                c0 = chunk * NCHUNK
                # -- load x^T slices, clamp, build fp16 features --
                F = {}
                if chunk == 0:
                    xch = xc0
                else:
                    xch = xc_pool.tile([P, IB, NCHUNK], f32, tag="xc", name="xc")
                    nc.sync.dma_start(xch[:], xt_r[:, :, c0:c0 + NCHUNK])
                nc.vector.tensor_scalar(xch[:], xch[:], -1.0, 1.0, Op.max, Op.min)
                for ib in range(IB):
                    xcb = xch[:, ib]

                    def tmp(tag, w=2, bufs=2):
                        return tmp_pool.tile([P, w * NCHUNK], f16, tag=tag, name=tag, bufs=bufs)

                    def fpair(f):
                        # (128, 2*NCHUNK) tile holding K-tiles (ib, f) on the
                        # left half and (ib, f+5) on the right half
                        t = f_pool.tile([P, 2 * NCHUNK], f16, tag=f"F_{ib}_{f}",
                                        name=f"F_{ib}_{f}")
                        F[ib, f] = t
                        return t

                    N = NCHUNK
                    # cumulative masks on GpSimd (1-input ops run near line-rate there)
                    cB = tmp("cB", 1); nc.gpsimd.tensor_scalar(cB[:], xcb[:], 0.0, None, Op.is_ge)
                    cD = tmp("cD", 1, 1); nc.gpsimd.tensor_scalar(cD[:], xcb[:], 1.0, None, Op.is_ge)
                    # window masks (exact 0/1 in fp16): Fm = [mL | mR]
                    Fm = fpair(0)
                    nc.gpsimd.tensor_scalar(Fm[:, :N], xcb[:], 0.0, None, Op.is_lt)
                    nc.vector.tensor_tensor(Fm[:, N:], cB[:], cD[:], Op.subtract)
                    # u-chain: ACT writes both halves from the same xcb
                    U = tmp("U")
                    nc.scalar.activation(U[:, :N], xcb[:], Act.Copy, bias=0.5)
                    nc.scalar.activation(U[:, N:], xcb[:], Act.Copy, bias=-0.5)
                    U2 = tmp("U2")
                    nc.scalar.activation(U2[:, :N], xcb[:], Act.Square, bias=b05[:])
                    nc.scalar.activation(U2[:, N:], xcb[:], Act.Square, bias=bm05[:])
                    U3 = tmp("U3")
                    nc.vector.tensor_tensor(U3[:], U2[:], U[:], Op.mult)
                    # windowed monomials: one 512-wide fp16 op per L/R pair
                    nc.vector.tensor_tensor(fpair(1)[:], Fm[:], U[:], Op.mult)
                    nc.vector.tensor_tensor(fpair(2)[:], Fm[:], U2[:], Op.mult)
                    nc.vector.tensor_tensor(fpair(3)[:], Fm[:], U3[:], Op.mult)
                    # knot-side features: g_H * u^3 == m_H * relu(u^3), fused
                    nc.vector.scalar_tensor_tensor(fpair(4)[:], U3[:], 0.0, Fm[:],
                                                   Op.max, Op.mult)
                    fs = f_pool.tile([P, NCHUNK], f16, tag=f"F_{ib}_s", name=f"F_{ib}_s")
                    F[ib, 10] = fs
                    nc.scalar.activation(fs[:], xcb[:], Act.Silu)

                # -- matmuls. Chunk 0 runs k-major over both 128-batch
                # subtiles so each weight slab feeds two matmuls the moment its
                # DMA lands (PE stays ahead of the initial weight stream);
                # later chunks run the subtiles serially so group-0's eviction
                # overlaps group-1's matmuls --
                def lhs(ib, f, ns):
                    if f == 10:
                        return F[ib, 10][:, ns * P:(ns + 1) * P]
                    if f < 5:
                        return F[ib, f][:, ns * P:(ns + 1) * P]
                    return F[ib, f - 5][:, NCHUNK + ns * P:NCHUNK + (ns + 1) * P]

                def evict(ps, ns):
                    o = out_pool.tile([P, O_SHARD], f32, tag="out", name="outt")
                    nc.vector.tensor_tensor(o[:], ps[:], bias_s[:], Op.add)
                    r0 = c0 + ns * P
                    nc.sync.dma_start(y_d[r0:r0 + P, :], o[:])

                if chunk == 0:
                    pss = [psum_out.tile([P, O_SHARD], f32, tag=f"psout{ns}",
                                         name=f"psout{ns}", bufs=2)
                           for ns in range(NSUB)]
                    for k, (ib, f) in enumerate(
                            (ib, f) for ib in range(IB) for f in range(NF)):
                        for ns in range(NSUB):
                            nc.tensor.matmul(
                                pss[ns][:], lhs(ib, f, ns), wt[ib][:, f],
                                start=(k == 0), stop=(k == KT - 1))
                    for ns in range(NSUB):
                        evict(pss[ns], ns)
                else:
                    for ns in range(NSUB):
                        ps = psum_out.tile([P, O_SHARD], f32, tag=f"psout{ns}",
                                           name=f"psout{ns}", bufs=2)
                        for k, (ib, f) in enumerate(
                                (ib, f) for ib in range(IB) for f in range(NF)):
                            nc.tensor.matmul(
                                ps[:], lhs(ib, f, ns), wt[ib][:, f],
                                start=(k == 0), stop=(k == KT - 1))
                        evict(ps, ns)

    nc.compile()
    return nc


def _fold_weights(coeff, w_base):
    """Fold the feature->basis matrix into coeff; returns (K, D_OUT) fp16."""
    M = _M48 / 48.0
    c64 = np.asarray(coeff).astype(np.float64)
    # Wf[f, i, o] = sum_b M[f, b] * coeff[o, i, b]
    Wf = np.einsum('fb,oib->fio', M, c64)
    W11 = np.concatenate([Wf, np.asarray(w_base).astype(np.float64).T[None]], axis=0)  # (11, i, o)
    # pack K as (ib, f, p): row k = ib*(NF*P) + f*P + p  <->  W11[f, ib*P+p, o]
    Wt = W11.reshape(NF, IB, P, D_OUT).transpose(1, 0, 2, 3).reshape(KT * P, D_OUT)
    return Wt.astype(np.float16)


def kernel(x, coeff, w_base, bias):
    global _PROGRAM
    from concourse.bass_utils import run_bass_kernel_spmd

    if _PROGRAM is None:
        _PROGRAM = _build_program()
    nc = _PROGRAM

    x = np.asarray(x, dtype=np.float32)
    Wt = _fold_weights(coeff, w_base)
    bias = np.asarray(bias, dtype=np.float32)

    in_maps = []
    for core in range(8):
        cn, co = divmod(core, MESH_O)
        in_maps.append({
            "xt": np.ascontiguousarray(x[cn * N_SHARD:(cn + 1) * N_SHARD].T),
            "wt": np.ascontiguousarray(Wt[:, co * O_SHARD:(co + 1) * O_SHARD]),
            "biasb": np.ascontiguousarray(np.broadcast_to(
                bias[co * O_SHARD:(co + 1) * O_SHARD], (P, O_SHARD)).astype(np.float32)),
        })

    res = run_bass_kernel_spmd(nc, in_maps, list(range(8)))

    y = np.empty((N_FULL, D_OUT), dtype=np.float32)
    for core in range(8):
        cn, co = divmod(core, MESH_O)
        y[cn * N_SHARD:(cn + 1) * N_SHARD, co * O_SHARD:(co + 1) * O_SHARD] = \
            res.results[core]["y"]
    return y


# revision 23
# speedup vs baseline: 1.0731x; 1.0154x over previous
"""BSplineKAN layer kernel for 8 Trainium2 NeuronCores.

Math
----
The reference computes, per element x = clip(x, -1, 1):
    y[n,o] = sum_{i,b} basis_b(x[n,i]) * coeff[o,i,b]  +  silu(x) @ w_base.T + bias
where basis is the 7-function clamped cubic B-spline basis on knots
{-1(x4), -0.5, 0, 0.5, 1(x4)}.  A quirk of the reference recurrence: at
x == 1.0 exactly (all clamped x >= 1 inputs) the basis row is all ZERO.

On [-1, 1) the basis functions are C^2 piecewise cubics with breakpoints at
+-0.5; we represent them exactly in a two-window local feature basis: for
each half H in {L: [-1,0), R: [0,1)} with center c_H = -+0.5, u = x - c_H,
window mask m_H, and knot-side mask g_H:
    feats_H = [m_H, m_H*u, m_H*u^2, m_H*u^3, g_H*u^3]
All ten features vanish at x == 1 (masks exclude it), reproducing the
reference's edge behavior exactly.  basis_b = M[f,b] @ feats (M integer/48,
exact).  M is folded into coeff on the host and silu/w_base appended as an
11th feature, giving one fused fp16 matmul
    y[n,o] = sum_{i,f} F_f(x[n,i]) * W[f,i,o] + bias
with K = 11*1024 = 11264.  Features are local (|u| <= 0.5), so the
contraction has no large-term cancellation; fp16 operands with fp32 PSUM
accumulation give ~5e-4 scale-relative absmax error (validated vs fp64).
Masks are exact in fp16 and the u-chain rounds at most 3 times, so the
all-fp16 feature pipeline adds no measurable error.

Distribution: 4-way batch x 2-way d_out mesh over 8 cores.  Per core:
x arrives host-transposed as (1024, 2048) fp32 (transposing on host is part
of sharding and keeps TensorE free of transposes), W-shard (11264, 512)
fp16 stays resident in SBUF, output (2048, 512) fp32.  Features are
computed on DVE (fp16 chain, 2x/4x modes) + ACT (affine/square/silu), and
TensorE runs back-to-back 88-tile K-accumulations into PSUM.
"""

import numpy as np

# ---- problem constants (hardcoded per contract) ----
N_FULL, D_IN, D_OUT = 8192, 1024, 1024
MESH_N, MESH_O = 4, 2                 # 4-way batch x 2-way d_out
N_SHARD = N_FULL // MESH_N            # 2048
O_SHARD = D_OUT // MESH_O             # 512
P = 128
NF = 11                               # 10 spline features + silu
IB = D_IN // P                        # 8 i-blocks
KT = IB * NF                          # 88 K-tiles
NCHUNK = 256                          # batch cols per pipeline chunk
NSUB = NCHUNK // P                    # 2
CHUNKS = N_SHARD // NCHUNK            # 8

# basis_b = sum_f feats_f * M[f, b];  feats order:
# [mL, mL*uL, mL*uL^2, mL*uL^3, gL*uL^3, mR, mR*uR, mR*uR^2, mR*uR^3, gR*uR^3]
_M48 = np.array([
    [0,    12,   28,   8,    0,    0,    0],
    [0,   -72,   24,   48,   0,    0,    0],
    [0,    144, -240,  96,   0,    0,    0],
    [-384, 672, -352,  64,   0,    0,    0],
    [384, -768,  576, -256,  64,   0,    0],
    [0,    0,    0,    8,    28,   12,   0],
    [0,    0,    0,   -48,  -24,   72,   0],
    [0,    0,    0,    96,  -240,  144,  0],
    [0,    0,   -64,   192, -224,  96,   0],
    [0,    0,    64,  -256,  576, -768,  384],
], dtype=np.float64)

_PROGRAM = None  # compiled Bass program, built once


def _build_program():
    import concourse.mybir as mybir
    import concourse.tile as tile
    from concourse import bacc

    f32 = mybir.dt.float32
    f16 = mybir.dt.float16
    Op = mybir.AluOpType
    Act = mybir.ActivationFunctionType

    nc = bacc.Bacc("TRN2", target_bir_lowering=False, debug=False)
    xt_d = nc.dram_tensor("xt", [D_IN, N_SHARD], f32, kind="ExternalInput").ap()
    w_d = nc.dram_tensor("wt", [KT * P, O_SHARD], f16, kind="ExternalInput").ap()
    b_d = nc.dram_tensor("biasb", [P, O_SHARD], f32, kind="ExternalInput").ap()
    y_d = nc.dram_tensor("y", [N_SHARD, O_SHARD], f32, kind="ExternalOutput").ap()

    with tile.TileContext(nc) as tc:
        with (
            tc.tile_pool(name="const", bufs=1) as const_pool,
            tc.tile_pool(name="wt", bufs=1) as wt_pool,
            tc.tile_pool(name="feat", bufs=2) as f_pool,
            tc.tile_pool(name="xc", bufs=2) as xc_pool,
            tc.tile_pool(name="tmp", bufs=2) as tmp_pool,
            tc.tile_pool(name="out", bufs=1) as out_pool,
            tc.tile_pool(name="pso", bufs=4, space="PSUM") as psum_out,
        ):
            bias_s = const_pool.tile([P, O_SHARD], f32)
            nc.sync.dma_start(bias_s[:], b_d[:])
            # tiny dummy activations up front so both ACT table sets load
            # concurrently with the initial DMAs instead of on the first
            # feature's critical path
            warm = const_pool.tile([P, 1], f32, name="warm")
            nc.gpsimd.memset(warm[:], 0.0)
            nc.scalar.activation(warm[:], warm[:], Act.Copy, bias=0.0)
            nc.scalar.activation(warm[:], warm[:], Act.Square)
            nc.scalar.activation(warm[:], warm[:], Act.Silu)
            b05 = const_pool.tile([P, 1], f32, name="b05")
            nc.gpsimd.memset(b05[:], 0.5)
            bm05 = const_pool.tile([P, 1], f32, name="bm05")
            nc.gpsimd.memset(bm05[:], -0.5)

            # warm-up: ~120 tiny matmuls on a zeroed tile fill the initial
            # DMA wait so the PE clock (HAM) is at full rate and the pipeline
            # is hot when the first real matmul issues
            wz = const_pool.tile([P, P], f16, name="wz")
            nc.gpsimd.memset(wz[:], 0.0)
            pw = psum_out.tile([P, 64], f32, tag="pwarm", name="pwarm")
            for i in range(120):
                nc.tensor.matmul(pw[:], wz[:], wz[:, :64],
                                 start=(i == 0), stop=(i == 119))

            # chunk-0 x load goes first so features can start while Wt streams in
            xt_r = xt_d.rearrange("(ib p) n -> p ib n", p=P)
            xc0 = xc_pool.tile([P, IB, NCHUNK], f32, tag="xc", name="xc0")
            nc.sync.dma_start(xc0[:], xt_r[:, :, 0:NCHUNK])

            # one DMA per ib-slab of 11 weight tiles: HWDGE charges per DMA
            # instruction, so batching is what keeps the queue off the
            # critical path during the initial weight stream
            wt = {}
            for ib in range(IB):
                t = wt_pool.tile([P, NF, O_SHARD], f16, tag=f"wt_{ib}", name=f"wt_{ib}")
                r0 = ib * NF * P
                nc.sync.dma_start(
                    t[:], w_d[r0:r0 + NF * P, :].rearrange("(f p) o -> p f o", p=P))
                wt[ib] = t

            for chunk in range(CHUNKS):
                c0 = chunk * NCHUNK
                # -- load x^T slices, clamp, build fp16 features --
                F = {}
                if chunk == 0:
                    xch = xc0
                else:
                    xch = xc_pool.tile([P, IB, NCHUNK], f32, tag="xc", name="xc")
                    nc.sync.dma_start(xch[:], xt_r[:, :, c0:c0 + NCHUNK])
                nc.vector.tensor_scalar(xch[:], xch[:], -1.0, 1.0, Op.max, Op.min)
                for ib in range(IB):
                    xcb = xch[:, ib]

                    def tmp(tag, w=2, bufs=2):
                        return tmp_pool.tile([P, w * NCHUNK], f16, tag=tag, name=tag, bufs=bufs)

                    def fpair(f):
                        # (128, 2*NCHUNK) tile holding K-tiles (ib, f) on the
                        # left half and (ib, f+5) on the right half
                        t = f_pool.tile([P, 2 * NCHUNK], f16, tag=f"F_{ib}_{f}",
                                        name=f"F_{ib}_{f}")
                        F[ib, f] = t
                        return t

                    N = NCHUNK
                    # cumulative masks on GpSimd (1-input ops run near line-rate there)
                    cB = tmp("cB", 1); nc.gpsimd.tensor_scalar(cB[:], xcb[:], 0.0, None, Op.is_ge)
                    cD = tmp("cD", 1, 1); nc.gpsimd.tensor_scalar(cD[:], xcb[:], 1.0, None, Op.is_ge)
                    # window masks (exact 0/1 in fp16): Fm = [mL | mR]
                    Fm = fpair(0)
                    nc.gpsimd.tensor_scalar(Fm[:, :N], xcb[:], 0.0, None, Op.is_lt)
                    nc.vector.tensor_tensor(Fm[:, N:], cB[:], cD[:], Op.subtract)
                    # u-chain: ACT writes both halves from the same xcb
                    U = tmp("U")
                    nc.scalar.activation(U[:, :N], xcb[:], Act.Copy, bias=0.5)
                    nc.scalar.activation(U[:, N:], xcb[:], Act.Copy, bias=-0.5)
                    U2 = tmp("U2")
                    nc.scalar.activation(U2[:, :N], xcb[:], Act.Square, bias=b05[:])
                    nc.scalar.activation(U2[:, N:], xcb[:], Act.Square, bias=bm05[:])
                    U3 = tmp("U3")
                    nc.vector.tensor_tensor(U3[:], U2[:], U[:], Op.mult)
                    # windowed monomials: one 512-wide fp16 op per L/R pair
                    nc.vector.tensor_tensor(fpair(1)[:], Fm[:], U[:], Op.mult)
                    nc.vector.tensor_tensor(fpair(2)[:], Fm[:], U2[:], Op.mult)
                    nc.vector.tensor_tensor(fpair(3)[:], Fm[:], U3[:], Op.mult)
                    # knot-side features: g_H * u^3 == m_H * relu(u^3), fused
                    nc.vector.scalar_tensor_tensor(fpair(4)[:], U3[:], 0.0, Fm[:],
                                                   Op.max, Op.mult)
                    fs = f_pool.tile([P, NCHUNK], f16, tag=f"F_{ib}_s", name=f"F_{ib}_s")
                    F[ib, 10] = fs
                    nc.scalar.activation(fs[:], xcb[:], Act.Silu)

                # -- matmuls. Chunk 0 runs k-major over both 128-batch
                # subtiles so each weight slab feeds two matmuls the moment its
                # DMA lands (PE stays ahead of the initial weight stream);
                # later chunks run the subtiles serially so group-0's eviction
                # overlaps group-1's matmuls --
                def lhs(ib, f, ns):
                    if f == 10:
                        return F[ib, 10][:, ns * P:(ns + 1) * P]
                    if f < 5:
                        return F[ib, f][:, ns * P:(ns + 1) * P]
                    return F[ib, f - 5][:, NCHUNK + ns * P:NCHUNK + (ns + 1) * P]

                def evict(ps, ns):
                    o = out_pool.tile([P, O_SHARD], f32, tag="out", name="outt")
                    nc.vector.tensor_tensor(o[:], ps[:], bias_s[:], Op.add)
                    r0 = c0 + ns * P
                    nc.sync.dma_start(y_d[r0:r0 + P, :], o[:])

                if chunk == 0:
                    pss = [psum_out.tile([P, O_SHARD], f32, tag=f"psout{ns}",
                                         name=f"psout{ns}", bufs=2)
                           for ns in range(NSUB)]
                    for k, (ib, f) in enumerate(
                            (ib, f) for ib in range(IB) for f in range(NF)):
                        for ns in range(NSUB):
                            nc.tensor.matmul(
                                pss[ns][:], lhs(ib, f, ns), wt[ib][:, f],
                                start=(k == 0), stop=(k == KT - 1))
                    for ns in range(NSUB):
                        evict(pss[ns], ns)
                else:
                    for ns in range(NSUB):
                        ps = psum_out.tile([P, O_SHARD], f32, tag=f"psout{ns}",
                                           name=f"psout{ns}", bufs=2)
                        for k, (ib, f) in enumerate(
                                (ib, f) for ib in range(IB) for f in range(NF)):
                            nc.tensor.matmul(
                                ps[:], lhs(ib, f, ns), wt[ib][:, f],
                                start=(k == 0), stop=(k == KT - 1))
                        evict(ps, ns)

    nc.compile()
    return nc


def _fold_weights(coeff, w_base):
    """Fold the feature->basis matrix into coeff; returns (K, D_OUT) fp16."""
    M = _M48 / 48.0
    c64 = np.asarray(coeff).astype(np.float64)
    # Wf[f, i, o] = sum_b M[f, b] * coeff[o, i, b]
    Wf = np.einsum('fb,oib->fio', M, c64)
    W11 = np.concatenate([Wf, np.asarray(w_base).astype(np.float64).T[None]], axis=0)  # (11, i, o)
    # pack K as (ib, f, p): row k = ib*(NF*P) + f*P + p  <->  W11[f, ib*P+p, o]
    Wt = W11.reshape(NF, IB, P, D_OUT).transpose(1, 0, 2, 3).reshape(KT * P, D_OUT)
    return Wt.astype(np.float16)


def kernel(x, coeff, w_base, bias):
    global _PROGRAM
    from concourse.bass_utils import run_bass_kernel_spmd

    if _PROGRAM is None:
        _PROGRAM = _build_program()
    nc = _PROGRAM

    x = np.asarray(x, dtype=np.float32)
    Wt = _fold_weights(coeff, w_base)
    bias = np.asarray(bias, dtype=np.float32)

    in_maps = []
    for core in range(8):
        cn, co = divmod(core, MESH_O)
        in_maps.append({
            "xt": np.ascontiguousarray(x[cn * N_SHARD:(cn + 1) * N_SHARD].T),
            "wt": np.ascontiguousarray(Wt[:, co * O_SHARD:(co + 1) * O_SHARD]),
            "biasb": np.ascontiguousarray(np.broadcast_to(
                bias[co * O_SHARD:(co + 1) * O_SHARD], (P, O_SHARD)).astype(np.float32)),
        })

    res = run_bass_kernel_spmd(nc, in_maps, list(range(8)))

    y = np.empty((N_FULL, D_OUT), dtype=np.float32)
    for core in range(8):
        cn, co = divmod(core, MESH_O)
        y[cn * N_SHARD:(cn + 1) * N_SHARD, co * O_SHARD:(co + 1) * O_SHARD] = \
            res.results[core]["y"]
    return y
